# revision 18
# baseline (speedup 1.0000x reference)
"""Trainium2 Bass kernel for nn_DemographicParityGap.

reference:
    class_sums[c, s] = sum_{n: bp[n]==c} output[n, s]        # segment sum, [C, S]
    demP = class_sums / output.sum(0)                        # [C, S]
    loss = mean over (c, pairs) of (demP[:, i0] - demP[:, i1])**2
    return -loss

Strategy (data-parallel over the 8 NeuronCores, hint-compliant):
  - Shard N rows across 8 cores.  Each core computes a partial per-(class,
    subgroup) sum; column sums are recovered as class_sums.sum(0) (every row
    belongs to exactly one class), so only one tiny [128, 160] partial per
    core leaves the device.  The host sums the 8 partials (the "all-reduce"
    of the tiny tensor) and finishes the pairwise-gap math.

  Device-side segment sum via one-hot matmuls, batched 16 row-groups per
  matmul so the PE stays off the instruction-issue floor:
    - x layout [128, T*8]: partition p holds rows (p*T + t), t<T, each
      row's 8 subgroup values contiguous.
    - one-hot tile [128, T*10] built by a single DVE is_equal against an
      iota constant packed next to bp in the same preloaded tensor.
    - per 16-group supergroup j: matmul(lhsT = x[:, 128j:128(j+1)] (16
      groups x 8 subgroups), rhs = onehot[:, 160j:160(j+1)] (16 groups x
      10 classes)) -> PSUM [128, 160].  Diagonal 8x10 blocks (t==g) are the
      per-class partial sums; off-diagonal blocks are ignored.  All
      supergroups accumulate into one PSUM tile (start on first, stop on
      last), drained once per core.

  This toolchain's walrus codegen allows exactly ONE sync-wait command per
  instruction (TT/LW/DMA structs alike), which dictates the sync shape:
    - bp+iota preloaded in one DMA; all is_equal ops share that single
      observed dependency.
    - a tiny DVE "observer" copy re-reads the newest one-hot tile before
      each is_equal so the is_equal carries only the PE buffer-release wait.
    - a 1x1 dummy matmul reading only x absorbs the x-DMA wait, so the
      first real matmul of a tile waits only on the DVE one-hot.
    - at most 8 DMAs total (1 bp + NX x-chunks + 1 out), one per DMAHW sem
      lane, so no DMA carries a lane-reuse wait on top of a data wait.
"""

import numpy as np

P = 128          # partitions
C = 10           # num classes
S = 8            # num subgroups
G = 16           # row-groups (of 128 rows each) per matmul; G*S == 128
NCORES = 8

N_FULL = 4_194_304
T = 256          # row-groups per partition per compute tile
NT = 16          # compute tiles per core; R = NT*P*T rows per core
NX = 4           # x-chunk DMAs per core (NT % NX == 0)


def build_nc(R, T, NT, NX):
    """Raw-Bass (no TileContext) pipeline.

    This walrus build allows exactly ONE sync-wait command per instruction;
    Tile's auto-sems routinely embed several (and its tail drain aggregates
    all procs), which fails codegen.  Raw Bass emits every wait as its own
    standalone instruction, which is always legal.

    Engine programs:
      SP (sync):  bp DMA, NX x-chunk DMAs (each -> own sem), final out DMA.
      DVE:        per tile: is_equal one-hot into half of a double buffer,
                  gated on bp DMA and (for reuse) PE tile completions;
                  final PSUM->SBUF drain copy.
      PE:         per tile: J matmuls accumulating into one PSUM tile,
                  gated on the x chunk's DMA sem and the DVE one-hot sem.
    """
    import concourse.bass as bass
    from concourse import mybir

    assert R == NT * P * T
    assert T % G == 0 and NT % NX == 0
    J = T // G
    W = T + C        # packed bp tile width: [bp(T), iota(C)]
    TPC = NT // NX   # tiles per x chunk
    f32 = mybir.dt.float32

    nc = bass.Bass()
    x = nc.dram_tensor("x", [R, S], f32, kind="ExternalInput")
    bpk = nc.dram_tensor("bp", [P, NT * W], f32, kind="ExternalInput")
    out = nc.dram_tensor("out", [P, G * C], f32, kind="ExternalOutput")

    # chunk k, partition p, u in [0, TPC*T): row = ((k*P)+p)*TPC*T + u
    x_r = x[:].rearrange("(k p u) s -> k p (u s)", k=NX, p=P)
    CHW = TPC * T * S     # x_all columns per chunk

    with (
        nc.sbuf_tensor([P, NT * T * S], f32) as x_all,
        nc.sbuf_tensor([P, NT * W], f32) as bp_all,
        nc.sbuf_tensor([P, 2 * T * C], f32) as oh2,
        nc.sbuf_tensor([P, G * C], f32) as out_sb,
        nc.psum_tensor([P, G * C], f32) as psum_t,
        nc.semaphore("s_bp") as s_bp,
        nc.semaphore("s_x0") as s_x0,
        nc.semaphore("s_x1") as s_x1,
        nc.semaphore("s_x2") as s_x2,
        nc.semaphore("s_x3") as s_x3,
        nc.semaphore("s_oh") as s_oh,
        nc.semaphore("s_pe") as s_pe,
        nc.Block() as block,
    ):
        s_x = [s_x0, s_x1, s_x2, s_x3][:NX]
        assert NX <= 4

        @block.sync
        def _(sync):
            sync.dma_start(out=bp_all[:], in_=bpk[:]).then_inc(s_bp, 16)
            for k in range(NX):
                sync.dma_start(
                    out=x_all[:, k * CHW:(k + 1) * CHW], in_=x_r[k]
                ).then_inc(s_x[k], 16)
            sync.wait_ge(s_oh, NT + 1)
            sync.dma_start(out=out[:], in_=out_sb[:]).then_inc(s_bp, 16)

        @block.vector
        def _(vector):
            vector.wait_ge(s_bp, 16)
            for i in range(NT):
                if i >= 2:
                    # oh half (i % 2) is reused: wait for tile i-2's matmuls
                    vector.wait_ge(s_pe, i - 1)
                bp_ap = bp_all[:, i * W:i * W + T]
                bp_bcast = bass.AP(
                    tensor=bp_ap.tensor,
                    offset=bp_ap.offset,
                    ap=[bp_ap.ap[0], [bp_ap.ap[1][0], T], [0, C]],
                )
                io_ap = bp_all[:, i * W + T:i * W + T + C]
                io_bcast = bass.AP(
                    tensor=io_ap.tensor,
                    offset=io_ap.offset,
                    ap=[io_ap.ap[0], [0, T], io_ap.ap[1]],
                )
                half = (i % 2) * T * C
                oh3 = oh2[:, half:half + T * C].rearrange(
                    "p (t c) -> p t c", t=T, c=C)
                vector.tensor_tensor(
                    out=oh3, in0=bp_bcast, in1=io_bcast,
                    op=mybir.AluOpType.is_equal,
                ).then_inc(s_oh, 1)
            vector.wait_ge(s_pe, NT)
            vector.tensor_copy(out=out_sb[:], in_=psum_t[:]).then_inc(s_oh, 1)

        @block.tensor
        def _(tensor):
            for i in range(NT):
                if i % TPC == 0:
                    tensor.wait_ge(s_x[i // TPC], 16)
                tensor.wait_ge(s_oh, i + 1)
                xcol = i * T * S
                half = (i % 2) * T * C
                for j in range(J):
                    first = i == 0 and j == 0
                    last = i == NT - 1 and j == J - 1
                    mm = tensor.matmul(
                        out=psum_t[:],
                        lhsT=x_all[:, xcol + j * (G * S):
                                   xcol + (j + 1) * (G * S)],
                        rhs=oh2[:, half + j * (G * C):
                                half + (j + 1) * (G * C)],
                        start=first, stop=last,
                    )
                    if j == J - 1:
                        mm.then_inc(s_pe, 1)
    return nc


_CACHE = {}


def _get_nc(R, T, NT, NX):
    key = (R, T, NT, NX)
    if key not in _CACHE:
        _CACHE[key] = build_nc(R, T, NT, NX)
    return _CACHE[key]


def pack_bp(bpf_shard, T, NT, NX):
    """[R] f32 -> [P, NT*(T+C)] f32 matching the chunked x layout.

    x slot (p, i*T + t) holds row ((k*P)+p)*(NT/NX)*T + d*T + t where
    i = k*(NT/NX) + d; bp must use the same permutation, with iota(C)
    appended per compute tile.
    """
    R = bpf_shard.shape[0]
    assert R == NT * P * T
    perm = bpf_shard.reshape(NX, P, (NT // NX) * T).transpose(1, 0, 2)
    perm = perm.reshape(P, NT, T)
    out = np.empty((P, NT, T + C), np.float32)
    out[:, :, :T] = perm
    out[:, :, T:] = np.arange(C, dtype=np.float32)
    return np.ascontiguousarray(out.reshape(P, NT * (T + C)))


def finish_host(partials):
    """partials: list of [P, G*C] f32 per-core PSUM drains -> scalar loss."""
    acc = np.zeros((P, G * C), np.float64)
    for r in partials:
        acc += r.astype(np.float64)
    cs_T = np.zeros((S, C), np.float64)
    for j in range(G):
        cs_T += acc[j * S:(j + 1) * S, j * C:(j + 1) * C]
    class_sums = cs_T.T                      # [C, S]
    colsum = class_sums.sum(axis=0)          # == output.sum(0)
    demP = class_sums / colsum
    i0, i1 = np.triu_indices(S, k=1)
    dpgs = (demP[:, i0] - demP[:, i1]) ** 2
    loss = dpgs.sum() / (C * i0.shape[0])
    return np.asarray(-loss, dtype=np.float32)


def run_device(x, bpf, trace=False, **trace_kwargs):
    """x: [N, 8] f32, bpf: [N] f32 (integer-valued). Returns BassKernelResults."""
    from concourse.bass_utils import run_bass_kernel_spmd

    N = x.shape[0]
    assert N % (NCORES * P * T) == 0, N
    R = N // NCORES
    NT_ = R // (P * T)
    in_maps = [
        {"x": x[c * R:(c + 1) * R],
         "bp": pack_bp(bpf[c * R:(c + 1) * R], T, NT_, NX)}
        for c in range(NCORES)
    ]
    nc = _get_nc(R, T, NT_, NX)
    return run_bass_kernel_spmd(
        nc, in_maps, core_ids=list(range(NCORES)), trace=trace, **trace_kwargs
    )


def kernel(output, biased_predictions, labels=None, num_classes=10,
           num_subgroups=8, **_ignored):
    assert int(num_classes) == C and int(num_subgroups) == S
    x = np.ascontiguousarray(np.asarray(output), dtype=np.float32)
    bp = np.asarray(biased_predictions)
    bpf = np.ascontiguousarray(bp.astype(np.float32))
    res = run_device(x, bpf)
    return finish_host([r["out"] for r in res.results])


# revision 20
# speedup vs baseline: 1.1395x; 1.1395x over previous
"""Trainium2 Bass kernel for nn_DemographicParityGap.

reference:
    class_sums[c, s] = sum_{n: bp[n]==c} output[n, s]        # segment sum, [C, S]
    demP = class_sums / output.sum(0)                        # [C, S]
    loss = mean over (c, pairs) of (demP[:, i0] - demP[:, i1])**2
    return -loss

Strategy (data-parallel over the 8 NeuronCores, hint-compliant):
  - Shard N rows across 8 cores.  Each core computes a partial per-(class,
    subgroup) sum; column sums are recovered as class_sums.sum(0) (every row
    belongs to exactly one class), so only one tiny [128, 160] partial per
    core leaves the device.  The host sums the 8 partials (the "all-reduce"
    of the tiny tensor) and finishes the pairwise-gap math.

  Device-side segment sum via one-hot matmuls, batched 16 row-groups per
  matmul so the PE stays off the instruction-issue floor:
    - x layout [128, T*8]: partition p holds rows (p*T + t), t<T, each
      row's 8 subgroup values contiguous.
    - one-hot tile [128, T*10] built by a single DVE is_equal against an
      iota constant packed next to bp in the same preloaded tensor.
    - per 16-group supergroup j: matmul(lhsT = x[:, 128j:128(j+1)] (16
      groups x 8 subgroups), rhs = onehot[:, 160j:160(j+1)] (16 groups x
      10 classes)) -> PSUM [128, 160].  Diagonal 8x10 blocks (t==g) are the
      per-class partial sums; off-diagonal blocks are ignored.  All
      supergroups accumulate into one PSUM tile (start on first, stop on
      last), drained once per core.

  This toolchain's walrus codegen allows exactly ONE sync-wait command per
  instruction (TT/LW/DMA structs alike), which dictates the sync shape:
    - bp+iota preloaded in one DMA; all is_equal ops share that single
      observed dependency.
    - a tiny DVE "observer" copy re-reads the newest one-hot tile before
      each is_equal so the is_equal carries only the PE buffer-release wait.
    - a 1x1 dummy matmul reading only x absorbs the x-DMA wait, so the
      first real matmul of a tile waits only on the DVE one-hot.
    - at most 8 DMAs total (1 bp + NX x-chunks + 1 out), one per DMAHW sem
      lane, so no DMA carries a lane-reuse wait on top of a data wait.
"""

import numpy as np

P = 128          # partitions
C = 10           # num classes
S = 8            # num subgroups
G = 16           # row-groups (of 128 rows each) per matmul; G*S == 128
NCORES = 8

N_FULL = 4_194_304
T = 256          # row-groups per partition per compute tile
NT = 16          # compute tiles per core; R = NT*P*T rows per core
NX = 16          # x/bp-chunk DMAs per core (NT % NX == 0)


def build_nc(R, T, NT, NX):
    """Raw-Bass (no TileContext) pipeline.

    This walrus build allows exactly ONE sync-wait command per instruction;
    Tile's auto-sems routinely embed several (and its tail drain aggregates
    all procs), which fails codegen.  Raw Bass emits every wait as its own
    standalone instruction, which is always legal.

    Engine programs:
      SP (sync):  bp DMA, NX x-chunk DMAs (each -> own sem), final out DMA.
      DVE:        per tile: is_equal one-hot into half of a double buffer,
                  gated on bp DMA and (for reuse) PE tile completions;
                  final PSUM->SBUF drain copy.
      PE:         per tile: J matmuls accumulating into one PSUM tile,
                  gated on the x chunk's DMA sem and the DVE one-hot sem.
    """
    from contextlib import ExitStack

    import concourse.bass as bass
    from concourse import mybir

    assert R == NT * P * T
    assert T % G == 0 and NT % NX == 0
    J = T // G
    W = T + C        # packed bp tile width: [bp(T), iota(C)]
    TPC = NT // NX   # tiles per x chunk
    f32 = mybir.dt.float32

    nc = bass.Bass()
    x = nc.dram_tensor("x", [R, S], f32, kind="ExternalInput")
    bpk = nc.dram_tensor("bp", [P, NT * W], f32, kind="ExternalInput")
    out = nc.dram_tensor("out", [P, G * C], f32, kind="ExternalOutput")

    # chunk k, partition p, u in [0, TPC*T): row = ((k*P)+p)*TPC*T + u
    x_r = x[:].rearrange("(k p u) s -> k p (u s)", k=NX, p=P)
    CHW = TPC * T * S     # x_all columns per chunk
    BW = TPC * W          # bp_all columns per chunk

    with ExitStack() as ctx:
        x_all = ctx.enter_context(nc.sbuf_tensor([P, NT * T * S], f32))
        bp_all = ctx.enter_context(nc.sbuf_tensor([P, NT * W], f32))
        oh2 = ctx.enter_context(nc.sbuf_tensor([P, 2 * T * C], f32))
        out_sb = ctx.enter_context(nc.sbuf_tensor([P, G * C], f32))
        psum_t = ctx.enter_context(nc.psum_tensor([P, G * C], f32))
        s_bp = [ctx.enter_context(nc.semaphore(f"s_bp{k}")) for k in range(NX)]
        s_x = [ctx.enter_context(nc.semaphore(f"s_x{k}")) for k in range(NX)]
        s_oh = ctx.enter_context(nc.semaphore("s_oh"))
        s_pe = ctx.enter_context(nc.semaphore("s_pe"))
        block = ctx.enter_context(nc.Block(no_gpsimd_drain=True))

        @block.sync
        def _(sync):
            # Interleave small bp chunks with x chunks on the FIFO HWDGE
            # ring: the first compute tile's inputs land within ~3us instead
            # of after one monolithic bp DMA + first big x chunk.
            for k in range(NX):
                sync.dma_start(
                    out=bp_all[:, k * BW:(k + 1) * BW],
                    in_=bpk[:, k * BW:(k + 1) * BW],
                ).then_inc(s_bp[k], 16)
                sync.dma_start(
                    out=x_all[:, k * CHW:(k + 1) * CHW], in_=x_r[k]
                ).then_inc(s_x[k], 16)
            sync.wait_ge(s_oh, NT + 1)
            sync.dma_start(out=out[:], in_=out_sb[:]).then_inc(s_bp[0], 16)

        @block.vector
        def _(vector):
            for i in range(NT):
                if i % TPC == 0:
                    vector.wait_ge(s_bp[i // TPC], 16)
                if i >= 2:
                    # oh half (i % 2) is reused: wait for tile i-2's matmuls
                    vector.wait_ge(s_pe, i - 1)
                bp_ap = bp_all[:, i * W:i * W + T]
                bp_bcast = bass.AP(
                    tensor=bp_ap.tensor,
                    offset=bp_ap.offset,
                    ap=[bp_ap.ap[0], [bp_ap.ap[1][0], T], [0, C]],
                )
                io_ap = bp_all[:, i * W + T:i * W + T + C]
                io_bcast = bass.AP(
                    tensor=io_ap.tensor,
                    offset=io_ap.offset,
                    ap=[io_ap.ap[0], [0, T], io_ap.ap[1]],
                )
                half = (i % 2) * T * C
                oh3 = oh2[:, half:half + T * C].rearrange(
                    "p (t c) -> p t c", t=T, c=C)
                vector.tensor_tensor(
                    out=oh3, in0=bp_bcast, in1=io_bcast,
                    op=mybir.AluOpType.is_equal,
                ).then_inc(s_oh, 1)
            vector.wait_ge(s_pe, NT)
            vector.tensor_copy(out=out_sb[:], in_=psum_t[:]).then_inc(s_oh, 1)

        @block.tensor
        def _(tensor):
            for i in range(NT):
                if i % TPC == 0:
                    tensor.wait_ge(s_x[i // TPC], 16)
                tensor.wait_ge(s_oh, i + 1)
                xcol = i * T * S
                half = (i % 2) * T * C
                for j in range(J):
                    first = i == 0 and j == 0
                    last = i == NT - 1 and j == J - 1
                    mm = tensor.matmul(
                        out=psum_t[:],
                        lhsT=x_all[:, xcol + j * (G * S):
                                   xcol + (j + 1) * (G * S)],
                        rhs=oh2[:, half + j * (G * C):
                                half + (j + 1) * (G * C)],
                        start=first, stop=last,
                    )
                    if j == J - 1:
                        mm.then_inc(s_pe, 1)
    return nc


_CACHE = {}


def _get_nc(R, T, NT, NX):
    key = (R, T, NT, NX)
    if key not in _CACHE:
        _CACHE[key] = build_nc(R, T, NT, NX)
    return _CACHE[key]


def pack_bp(bpf_shard, T, NT, NX):
    """[R] f32 -> [P, NT*(T+C)] f32 matching the chunked x layout.

    x slot (p, i*T + t) holds row ((k*P)+p)*(NT/NX)*T + d*T + t where
    i = k*(NT/NX) + d; bp must use the same permutation, with iota(C)
    appended per compute tile.
    """
    R = bpf_shard.shape[0]
    assert R == NT * P * T
    perm = bpf_shard.reshape(NX, P, (NT // NX) * T).transpose(1, 0, 2)
    perm = perm.reshape(P, NT, T)
    out = np.empty((P, NT, T + C), np.float32)
    out[:, :, :T] = perm
    out[:, :, T:] = np.arange(C, dtype=np.float32)
    return np.ascontiguousarray(out.reshape(P, NT * (T + C)))


def finish_host(partials):
    """partials: list of [P, G*C] f32 per-core PSUM drains -> scalar loss."""
    acc = np.zeros((P, G * C), np.float64)
    for r in partials:
        acc += r.astype(np.float64)
    cs_T = np.zeros((S, C), np.float64)
    for j in range(G):
        cs_T += acc[j * S:(j + 1) * S, j * C:(j + 1) * C]
    class_sums = cs_T.T                      # [C, S]
    colsum = class_sums.sum(axis=0)          # == output.sum(0)
    demP = class_sums / colsum
    i0, i1 = np.triu_indices(S, k=1)
    dpgs = (demP[:, i0] - demP[:, i1]) ** 2
    loss = dpgs.sum() / (C * i0.shape[0])
    return np.asarray(-loss, dtype=np.float32)


def run_device(x, bpf, trace=False, **trace_kwargs):
    """x: [N, 8] f32, bpf: [N] f32 (integer-valued). Returns BassKernelResults."""
    from concourse.bass_utils import run_bass_kernel_spmd

    N = x.shape[0]
    assert N % (NCORES * P * T) == 0, N
    R = N // NCORES
    NT_ = R // (P * T)
    in_maps = [
        {"x": x[c * R:(c + 1) * R],
         "bp": pack_bp(bpf[c * R:(c + 1) * R], T, NT_, NX)}
        for c in range(NCORES)
    ]
    nc = _get_nc(R, T, NT_, NX)
    return run_bass_kernel_spmd(
        nc, in_maps, core_ids=list(range(NCORES)), trace=trace, **trace_kwargs
    )


def kernel(output, biased_predictions, labels=None, num_classes=10,
           num_subgroups=8, **_ignored):
    assert int(num_classes) == C and int(num_subgroups) == S
    x = np.ascontiguousarray(np.asarray(output), dtype=np.float32)
    bp = np.asarray(biased_predictions)
    bpf = np.ascontiguousarray(bp.astype(np.float32))
    res = run_device(x, bpf)
    return finish_host([r["out"] for r in res.results])
